# revision 9
# baseline (speedup 1.0000x reference)
"""Trainium2 Bass kernel for nn_CacheAttention (16-head causal MHA, T=2048 B=4 E=1024).

Sharding: 16 heads split across 8 NeuronCores (2 heads / core).  Each core:
  - projects q/k/v with its 128-column slice of wq/wk/wv (E-contraction on
    partitions, activations streamed in head-dim-transposed layout),
  - runs flash-style attention for its 8 (batch, head) pairs in
    transposed-score layout (scores^T = K_tile.T @ Q, softmax sum picked up
    for free via a ones-column appended to V),
  - applies its 128-row slice of wo, producing a partial [B*T, E] output.
Host sums the 8 partials and adds the output bias.

Perf structure: bf16 compute (f32 PSUM), the two heads' K=64 score matmuls
are emitted back-to-back so they run concurrently in disjoint PE row-groups,
score tiles are double-bank [128,1024] so each ACT exp covers two s-tiles,
softmax normalization is reciprocal(DVE) -> partition_broadcast(GpSimd) ->
multiply(DVE).  The additive mask is applied as exp(mask) multiplied into
the probabilities; when the mask is exactly the causal pattern the kernel
skips fully-masked tiles and uses 4 cached diagonal patterns.
"""

import sys

if "/opt/trn_rl_repo" not in sys.path:
    sys.path.insert(0, "/opt/trn_rl_repo")

import numpy as np
import ml_dtypes

import concourse.mybir as mybir
import concourse.tile as tile
from concourse import bacc
from concourse.bass_utils import run_bass_kernel_spmd
from concourse.masks import make_identity

BF16 = ml_dtypes.bfloat16
F32 = mybir.dt.float32
BF = mybir.dt.bfloat16

T, B, E = 2048, 4, 1024
H, D = 16, 64
NCORES = 8
HPC = H // NCORES          # heads per core = 2
DC = HPC * D               # head-dim columns per core = 128
R = B * T                  # rows (b-major: r = b*T + t) = 8192
KT = E // 128              # E contraction tiles = 8
NCH = T // 512             # q chunks per (b,h) pair = 4
NST = T // 128             # s tiles per (b,h) pair = 16
SCALE = float(D) ** -0.5

_CACHE = {}


def _build(causal: bool, reps: int = 1):
    nc = bacc.Bacc("TRN2", target_bir_lowering=False, debug=False, num_devices=NCORES)

    qT_d = nc.dram_tensor("qT", [E, R], BF, kind="ExternalInput")
    kT_d = nc.dram_tensor("kT", [E, R], BF, kind="ExternalInput")
    vT_d = nc.dram_tensor("vT", [E, R], BF, kind="ExternalInput")
    wqT_d = nc.dram_tensor("wqT", [E, DC], BF, kind="ExternalInput")
    wkT_d = nc.dram_tensor("wkT", [E, DC], BF, kind="ExternalInput")
    wvT_d = nc.dram_tensor("wvT", [E, DC], BF, kind="ExternalInput")
    woT_d = nc.dram_tensor("woT", [DC, E], BF, kind="ExternalInput")
    bq_d = nc.dram_tensor("bq", [DC, 1], F32, kind="ExternalInput")
    bk_d = nc.dram_tensor("bk", [DC, 1], F32, kind="ExternalInput")
    bv_d = nc.dram_tensor("bv", [DC, 1], F32, kind="ExternalInput")
    if causal:
        dm_d = nc.dram_tensor("dmask", [4, 128, 512], BF, kind="ExternalInput")
    else:
        em_d = nc.dram_tensor("emaskT", [T, T], BF, kind="ExternalInput")
    out_d = nc.dram_tensor("out", [R, E], BF, kind="ExternalOutput")

    Exp = mybir.ActivationFunctionType.Exp
    add = mybir.AluOpType.add
    mult = mybir.AluOpType.mult

    with tile.TileContext(nc) as tc:
        with (
            tc.tile_pool(name="wp", bufs=1) as wp,
            tc.tile_pool(name="mp", bufs=2) as mp,
            tc.tile_pool(name="ps", bufs=2, space="PSUM") as ps,
        ):
            # ---- constants / weights (persistent) ----
            wq_sb = wp.tile([128, KT, DC], BF, tag="wq")
            wk_sb = wp.tile([128, KT, DC], BF, tag="wk")
            wv_sb = wp.tile([128, KT, DC], BF, tag="wv")
            for w_sb, w_d in ((wq_sb, wqT_d), (wk_sb, wkT_d), (wv_sb, wvT_d)):
                nc.sync.dma_start(w_sb, w_d.ap().rearrange("(k p) d -> p k d", p=128))
            wo_sb = wp.tile([DC, E], BF, tag="wo")
            nc.sync.dma_start(wo_sb, woT_d.ap())
            bq_sb = wp.tile([DC, 1], F32, tag="bq")
            nc.sync.dma_start(bq_sb, bq_d.ap())
            bk_sb = wp.tile([DC, 1], F32, tag="bk")
            nc.sync.dma_start(bk_sb, bk_d.ap())
            bv_sb = wp.tile([DC, 1], F32, tag="bv")
            nc.sync.dma_start(bv_sb, bv_d.ap())
            ident = wp.tile([128, 128], BF, tag="ident")
            make_identity(nc, ident)
            if causal:
                dm_sb = wp.tile([128, 4 * 512], BF, tag="dm")
                nc.sync.dma_start(
                    dm_sb.rearrange("p (j q) -> p j q", q=512),
                    dm_d.ap().rearrange("j p q -> p j q"),
                )

            for b4 in range(B * reps):
                b = b4 % B
                # ---- projections for batch b (rows b*T .. b*T+T) ----
                qT_b = mp.tile([DC, T], BF, tag="qTb")
                kT_b = mp.tile([DC, T], BF, tag="kTb")
                vTt = mp.tile([DC, T], BF, tag="vTt")
                for (src_d, w_sb, bias, scale, dst) in (
                    (qT_d, wq_sb, bq_sb, SCALE, qT_b),
                    (kT_d, wk_sb, bk_sb, 1.0, kT_b),
                    (vT_d, wv_sb, bv_sb, 1.0, vTt),
                ):
                    xin = mp.tile([128, KT, T], BF, tag="xin")
                    for k in range(KT):
                        nc.sync.dma_start(
                            xin[:, k, :],
                            src_d.ap()[128 * k : 128 * (k + 1), b * T : (b + 1) * T],
                        )
                    # k-outer so the first matmul only waits on one input tile;
                    # two n-columns accumulate in parallel PSUM banks per pass.
                    for g in range(NCH // 2):
                        pps = [
                            ps.tile([128, 512], F32, tag="mm", name=f"pp{i}")
                            for i in range(2)
                        ]
                        for k in range(KT):
                            for i in range(2):
                                n = 2 * g + i
                                nc.tensor.matmul(
                                    pps[i],
                                    w_sb[:, k, :],
                                    xin[:, k, 512 * n : 512 * (n + 1)],
                                    start=(k == 0),
                                    stop=(k == KT - 1),
                                )
                        for i in range(2):
                            n = 2 * g + i
                            # (x + b) * s on DVE, psum f32 -> sbuf bf16
                            nc.vector.tensor_scalar(
                                dst[:, 512 * n : 512 * (n + 1)], pps[i], bias, scale,
                                add, mult,
                            )

                # ---- v^T -> v natural (+ones cols) via PE transpose ----
                # layout per s-tile j: [v_h0(64) | 1 | v_h1(64) | 1] = 130 cols
                v_nat = mp.tile([128, NST * 130], BF, tag="vnat")
                for j in range(NST):
                    pt = ps.tile([128, 128], BF, tag="mm")
                    nc.tensor.transpose(pt, vTt[:, 128 * j : 128 * (j + 1)], ident)
                    for h in range(HPC):
                        nc.any.tensor_copy(
                            v_nat[:, 130 * j + 65 * h : 130 * j + 65 * h + 64],
                            pt[:, 64 * h : 64 * h + 64],
                        )
                vv = v_nat.rearrange("p (r c) -> p r c", c=65)
                nc.vector.memset(vv[:, :, 64], 1.0)

                # ---- attention, both heads interleaved ----
                attnT_b = mp.tile([DC, T], BF, tag="attnTb")
                for c in range(NCH):
                    n_s = 4 * (c + 1) if causal else NST
                    a_ps = [
                        ps.tile([65, 512], F32, tag=f"at{h}", bufs=1, name=f"a_ps{h}")
                        for h in range(HPC)
                    ]
                    for jp in range(n_s // 2):
                        j0 = 2 * jp
                        sc = [
                            ps.tile([128, 1024], F32, tag=f"sc{h}", bufs=1, name=f"sc{h}")
                            for h in range(HPC)
                        ]
                        # QK: emit the two heads back-to-back per s-tile so the
                        # K=64 matmuls pack into disjoint PE row-groups.
                        for dj in range(2):
                            j = j0 + dj
                            for h in range(HPC):
                                hs = 64 * h
                                nc.tensor.matmul(
                                    sc[h][:, 512 * dj : 512 * (dj + 1)],
                                    kT_b[hs : hs + 64, 128 * j : 128 * (j + 1)],
                                    qT_b[hs : hs + 64, 512 * c : 512 * (c + 1)],
                                    start=True,
                                    stop=True,
                                )
                        em0 = None
                        if not causal:
                            em0 = mp.tile([128, 1024], BF, tag="em", bufs=3)
                            for dj in range(2):
                                nc.sync.dma_start(
                                    em0[:, 512 * dj : 512 * (dj + 1)],
                                    em_d.ap()[
                                        128 * (j0 + dj) : 128 * (j0 + dj + 1),
                                        512 * c : 512 * (c + 1),
                                    ],
                                )
                        elif j0 >= 4 * c:
                            em0 = dm_sb[:, 512 * (j0 - 4 * c) : 512 * (j0 - 4 * c) + 1024]
                        for h in range(HPC):
                            pT = mp.tile([128, 1024], BF, tag="pT", bufs=3)
                            nc.scalar.activation(pT, sc[h], Exp)
                            if em0 is not None:
                                pm = mp.tile([128, 1024], BF, tag="pm", bufs=2)
                                nc.vector.tensor_tensor(pm, pT, em0, mult)
                                pT = pm
                            for dj in range(2):
                                j = j0 + dj
                                nc.tensor.matmul(
                                    a_ps[h],
                                    v_nat[:, 130 * j + 65 * h : 130 * j + 65 * (h + 1)],
                                    pT[:, 512 * dj : 512 * (dj + 1)],
                                    start=(jp == 0 and dj == 0),
                                    stop=(jp == n_s // 2 - 1 and dj == 1),
                                )
                    for h in range(HPC):
                        hs = 64 * h
                        rl = mp.tile([1, 512], BF, tag="rl", bufs=2)
                        with nc.allow_low_precision(reason="softmax denom recip"):
                            nc.vector.reciprocal(rl, a_ps[h][64:65, :])
                        rlb = mp.tile([64, 512], BF, tag="rlb", bufs=2)
                        nc.gpsimd.partition_broadcast(rlb, rl)
                        nc.vector.tensor_tensor(
                            attnT_b[hs : hs + 64, 512 * c : 512 * (c + 1)],
                            a_ps[h][0:64, :],
                            rlb,
                            mult,
                        )

                # ---- output projection (partial; host sums cores) ----
                for r in range(T // 128):
                    o_sb = mp.tile([128, E], BF, tag="osb")
                    for n in range(E // 512):
                        o_ps = ps.tile([128, 512], F32, tag="mm")
                        nc.tensor.matmul(
                            o_ps,
                            attnT_b[:, 128 * r : 128 * (r + 1)],
                            wo_sb[:, 512 * n : 512 * (n + 1)],
                            start=True,
                            stop=True,
                        )
                        nc.any.tensor_copy(o_sb[:, 512 * n : 512 * (n + 1)], o_ps)
                    nc.sync.dma_start(
                        out_d.ap()[b * T + 128 * r : b * T + 128 * (r + 1), :], o_sb
                    )

    nc.compile()
    return nc


def _causal_mask_ref():
    return np.where(
        np.arange(T)[:, None] >= np.arange(T)[None, :], np.float32(0.0), np.float32(-1e9)
    ).astype(np.float32)


def _diag_patterns():
    # pattern[j, s, q] = 1.0 if (128*j + s) <= q else 0.0   (q in 0..511)
    j = np.arange(4)[:, None, None]
    s = np.arange(128)[None, :, None]
    q = np.arange(512)[None, None, :]
    return ((128 * j + s) <= q).astype(BF16)


def _prep_in_maps(query, key, value, attn_mask, wq, bq, wk, bk, wv, bv, wo, causal):
    # [T, B, E] -> [E, B*T] b-major columns, bf16
    qT = np.ascontiguousarray(query.transpose(2, 1, 0).reshape(E, R)).astype(BF16)
    kT = np.ascontiguousarray(key.transpose(2, 1, 0).reshape(E, R)).astype(BF16)
    vT = np.ascontiguousarray(value.transpose(2, 1, 0).reshape(E, R)).astype(BF16)
    common = {"qT": qT, "kT": kT, "vT": vT}
    if causal:
        common["dmask"] = np.ascontiguousarray(_diag_patterns())
    else:
        common["emaskT"] = np.exp(attn_mask.astype(np.float64).T).astype(BF16)
    in_maps = []
    for c in range(NCORES):
        sl = slice(DC * c, DC * (c + 1))
        m = dict(common)
        m["wqT"] = np.ascontiguousarray(wq[sl, :].T).astype(BF16)
        m["wkT"] = np.ascontiguousarray(wk[sl, :].T).astype(BF16)
        m["wvT"] = np.ascontiguousarray(wv[sl, :].T).astype(BF16)
        m["woT"] = np.ascontiguousarray(wo[:, sl].T).astype(BF16)
        m["bq"] = bq[sl].astype(np.float32)[:, None]
        m["bk"] = bk[sl].astype(np.float32)[:, None]
        m["bv"] = bv[sl].astype(np.float32)[:, None]
        in_maps.append(m)
    return in_maps


def _postprocess(results, bo):
    acc = results[0]["out"].astype(np.float32)
    for c in range(1, NCORES):
        acc = acc + results[c]["out"].astype(np.float32)
    out = acc.reshape(B, T, E).transpose(1, 0, 2) + bo[None, None, :]
    return np.ascontiguousarray(out.astype(np.float32))


def kernel(query, key, value, attn_mask, wq, bq, wk, bk, wv, bv, wo, bo):
    assert query.shape == (T, B, E), query.shape
    causal = bool(np.array_equal(attn_mask, _causal_mask_ref()))
    if causal not in _CACHE:
        _CACHE[causal] = _build(causal)
    nc = _CACHE[causal]
    in_maps = _prep_in_maps(
        query, key, value, attn_mask, wq, bq, wk, bk, wv, bv, wo, causal
    )
    res = run_bass_kernel_spmd(nc, in_maps, core_ids=list(range(NCORES)))
    return _postprocess(res.results, np.asarray(bo, dtype=np.float32))
